# revision 16
# baseline (speedup 1.0000x reference)
"""CrossSubgConv Trainium2 kernel.

out[b,i,j,d] = sum_k h2[b,i,k,d] * A[b,k,j]
  h1 = relu(X @ W1 + b1); h2 = relu(h1 @ W2 + b2)

Sharding: data-parallel over B=32 across 8 cores (4 graphs per core),
MLP weights replicated. No collectives.

Layout strategy (per core, BL=4 graphs, per graph 2 halves of 64 i's):

  Host pre-packs X into bf16 M1-layout [BL, 2, 128, 4096]:
    partition p = 64*par + d, free = (t, i4, k), i = 64h + 8t + 4par + i4
  so every DMA is a contiguous 8KB-per-partition copy, and the MLP1
  matmul can use a block-diagonal stationary diag(W1, W1) [128,128] to
  process two K=64 chunks per pass (full PE array at K=128).

  MLP1: ps1[128,512] = diag(W1,W1)^T @ M1[:, t]  (8 matmuls/half)
        drain: relu(ps1 + [b1;b1]) -> h1t bf16      (ACT or DVE)
  MLP2: stationary = h1t[:, (t,i4)] [128,128] (two i's stacked on
        partitions), rhs = diag(W2,W2) -> ps2[k, (i4,par,d)]
        drain: relu -> Hh [k, (iloc,d)] with (i4,par)->(iloc) reorder
  Agg:  lhsT = A[b] [k,j] stationary, rhs = Hh chunks -> psC [j,(iloc,d)]
        drain: copy -> outS bf16
  Store: outS [j, (i-half, d)] -> OUT[b, j, i0:i0+64, :]  (j-major DRAM
        layout; host un-transposes to [b, i, j, d] afterwards)

X loads ride SWDGE (gpsimd) so the Tile scheduler's RAW/WAR edges never
exceed the HWDGE DMA ISA sync-wait cap; stores ride the idle SP HWDGE
queue with a single same-engine RAW edge (the 4 agg drains of one
iteration all run on one engine, alternating per iteration).
"""

import numpy as np

B, N, D = 32, 128, 64
NCORES = 8
BL = B // NCORES  # graphs per core
OUT_BF16 = True   # store output bf16, upcast on host


# Drain-engine split (single writer per SBUF tile / per PSUM pool so no
# ACT or MM instruction ever exceeds its ISA sync-wait slots):
#   MLP1 (8x512) -> ACT into h1t; MLP2 even t -> ACT into HhA,
#   odd t -> DVE into HhD; agg (4x1024) -> DVE into outS.
# Balance: ACT 12*570ns = 6.8us/iter, DVE 4*658 + 4*1192 = 7.4us/iter.


def _build(has_b2: bool, out_bf16: bool = OUT_BF16):
    from concourse import bass, tile
    from concourse import mybir

    f32 = mybir.dt.float32
    bf16 = mybir.dt.bfloat16
    Relu = mybir.ActivationFunctionType.Relu
    Copy = mybir.ActivationFunctionType.Copy

    obt = bf16 if out_bf16 else f32

    nc = bass.Bass()

    Xd = nc.declare_dram_parameter("XT", [BL, 2, 128, 4096], bf16, isOutput=False)
    Ad = nc.declare_dram_parameter("A", [BL, N, N], bf16, isOutput=False)
    W1d = nc.declare_dram_parameter("W1D", [128, 128], bf16, isOutput=False)
    W2d = nc.declare_dram_parameter("W2D", [128, 128], bf16, isOutput=False)
    B1d = nc.declare_dram_parameter("B1R", [128, 1], f32, isOutput=False)
    B2d = nc.declare_dram_parameter("B2R", [1, 512], f32, isOutput=False)
    # [b, j, i, d] -- j-major; host transposes back
    Od = nc.declare_dram_parameter("OUT", [BL, N, N, D], obt, isOutput=True)

    with tile.TileContext(nc) as tc:
        with (
            tc.tile_pool(name="const", bufs=1) as cpool,
            tc.tile_pool(name="abuf", bufs=1) as apool,
            tc.tile_pool(name="m1", bufs=3) as m1pool,
            tc.tile_pool(name="h1", bufs=3) as h1pool,
            tc.tile_pool(name="hq", bufs=3) as hqpool,
            tc.tile_pool(name="outs", bufs=3) as opool,
            tc.tile_pool(name="ps1", bufs=2, space="PSUM") as ps1pool,
            tc.tile_pool(name="ps2", bufs=2, space="PSUM") as ps2pool,
            tc.tile_pool(name="psC", bufs=2, space="PSUM") as psCpool,
            tc.tile_pool(name="scr", bufs=1, space="PSUM") as scrpool,
        ):
            W1t = cpool.tile([128, 128], bf16)
            nc.sync.dma_start(out=W1t[:], in_=W1d[:])
            W2t = cpool.tile([128, 128], bf16)
            nc.sync.dma_start(out=W2t[:], in_=W2d[:])
            b1t = cpool.tile([128, 1], f32)
            nc.sync.dma_start(out=b1t[:], in_=B1d[:])

            if has_b2:
                ones_row = cpool.tile([1, 128], bf16)
                nc.vector.memset(ones_row[:], 1.0)
                b2f = cpool.tile([1, 512], f32)
                nc.sync.dma_start(out=b2f[:], in_=B2d[:])
                b2row = cpool.tile([1, 512], bf16)
                nc.vector.tensor_copy(b2row[:], b2f[:])

            # ACT pre-observes the bias load so MLP1 relus wait only on PE
            scrB = cpool.tile([1, 4], f32)
            nc.scalar.activation(scrB[:, 0:1], b1t[0:1, :], Copy)

            Abfs = []
            for b in range(BL):
                Abf = apool.tile([N, N], bf16, tag=f"abf{b}")
                nc.sync.dma_start(out=Abf[:], in_=Ad[b])
                Abfs.append(Abf)

            # PE observer scrap: every observer matmul writes here. The first
            # (warm) observer also absorbs the SP const-DMA semaphore so real
            # matmuls never carry it.
            scr = scrpool.tile([1, 1], f32)

            def observe(src_ap):
                nc.tensor.matmul(
                    scr[:], src_ap, src_ap, start=True, stop=True,
                    skip_group_check=True,
                )

            observe(Abfs[-1][0:1, 0:1])

            prev_hhd = None
            prev_outs = None

            for b in range(BL):
                Abf = Abfs[b]
                for h in range(2):
                    it = b * 2 + h

                    M1 = m1pool.tile([128, 4096], bf16)
                    nc.scalar.dma_start(out=M1[:], in_=Xd[b, h])
                    # absorb the load semaphore into PE once per iteration
                    observe(M1[0:1, 0:1])

                    # ---- MLP layer 1 (drain: ACT into h1t) ----
                    h1t = h1pool.tile([128, 4096], bf16)
                    for t in range(8):
                        sl = slice(512 * t, 512 * (t + 1))
                        ps1 = ps1pool.tile([128, 512], f32)
                        nc.tensor.matmul(
                            ps1[:], W1t[:], M1[:, sl], start=True, stop=True
                        )
                        nc.scalar.activation(
                            h1t[:, sl], ps1[:], Relu, bias=b1t[:]
                        )

                    # ---- MLP layer 2 (drain: ACT even t -> HhA, DVE odd -> HhD) ----
                    HhA = hqpool.tile([128, 2048], bf16, tag="hha")
                    HhD = hqpool.tile([128, 2048], bf16, tag="hhd")
                    for t in range(8):
                        act_t = t % 2 == 0
                        ps2 = ps2pool.tile([128, 512], f32)
                        if not act_t:
                            # ps2 slot's previous drain was DVE (t-2): absorb
                            # its semaphore so the first matmul waits only ACT
                            if t >= 3:
                                observe(HhD[0:1, 512 * ((t - 2) // 2) : 512 * ((t - 2) // 2) + 1])
                            elif prev_hhd is not None:
                                observe(prev_hhd[0:1, 512 * 3 : 512 * 3 + 1])
                        if has_b2:
                            nc.tensor.matmul(
                                ps2[:], ones_row[:], b2row[:],
                                start=True, stop=False, skip_group_check=True,
                            )
                        for i4 in range(4):
                            nc.tensor.matmul(
                                ps2[:, 128 * i4 : 128 * (i4 + 1)],
                                h1t[:, 512 * t + 128 * i4 : 512 * t + 128 * (i4 + 1)],
                                W2t[:],
                                start=not has_b2,
                                stop=(not has_b2) or i4 == 3,
                                skip_group_check=has_b2,
                            )
                        # psum cols (i4, par, d) -> Hh cols (iloc=4*par+i4, d)
                        dst = HhA if act_t else HhD
                        tl = t // 2
                        out_ap = dst[:, 512 * tl : 512 * (tl + 1)].rearrange(
                            "p (par i4 d) -> p i4 par d", par=2, i4=4
                        )
                        in_ap = ps2[:].rearrange(
                            "p (i4 par d) -> p i4 par d", par=2, i4=4
                        )
                        if act_t:
                            nc.scalar.activation(out_ap, in_ap, Relu)
                        else:
                            nc.vector.tensor_scalar_max(out_ap, in_ap, 0.0)
                    prev_hhd = HhD

                    # ---- cross-subgraph aggregation (drain: DVE into outS) ----
                    outS = opool.tile([128, 4096], obt)  # [j, (i_half, d)]
                    for t in range(8):
                        src = HhA if t % 2 == 0 else HhD
                        rhs = src[:, 512 * (t // 2) : 512 * (t // 2 + 1)]
                        psC = psCpool.tile([128, 512], f32)
                        # absorb the DVE copy semaphore of this slot's
                        # previous tenant (copy t-2, or prev iteration)
                        if t >= 2:
                            observe(outS[0:1, 512 * (t - 2) : 512 * (t - 2) + 1])
                        elif prev_outs is not None:
                            observe(prev_outs[0:1, 512 * (6 + t) : 512 * (6 + t) + 1])
                        nc.tensor.matmul(
                            psC[:], Abf[:], rhs, start=True, stop=True
                        )
                        nc.vector.tensor_copy(
                            outS[:, 512 * t : 512 * (t + 1)], psC[:]
                        )
                    prev_outs = outS

                    nc.sync.dma_start(
                        out=Od[b, :, 64 * h : 64 * (h + 1), :], in_=outS[:]
                    )

    # gen3 structs accept a single sync wait (EventSemaphore: two). Spill
    # extra matmul waits onto the preceding Ldweights, then split any
    # remaining multi-wait instructions via EventSemaphore insertions.
    from concourse import bacc
    bacc._bass_rust.move_matmul_waits_to_ldweights(nc.m)
    bacc._bass_rust.generate_event_semaphores(nc)

    return nc


_CACHE = {}


def _get_nc(has_b2):
    if has_b2 not in _CACHE:
        _CACHE[has_b2] = _build(has_b2)
    return _CACHE[has_b2]


def _pack_inputs(X, A, W1, b1, W2, b2):
    """Host-side packing into the device layouts (see module docstring)."""
    import ml_dtypes

    bf = ml_dtypes.bfloat16
    X = np.ascontiguousarray(X, dtype=np.float32)
    A = np.ascontiguousarray(A, dtype=np.float32)
    W1 = np.asarray(W1, np.float32)
    W2 = np.asarray(W2, np.float32)
    b1 = np.asarray(b1, np.float32)
    b2 = np.asarray(b2, np.float32)

    W1D = np.zeros((128, 128), np.float32)
    W1D[:64, :64] = W1
    W1D[64:, 64:] = W1
    W2D = np.zeros((128, 128), np.float32)
    W2D[:64, :64] = W2
    W2D[64:, 64:] = W2
    B1R = np.concatenate([b1, b1]).reshape(128, 1)
    # ps2 cols are (i4, par, d): b2 pattern tiles (par, d) four times
    B2R = np.tile(np.concatenate([b2, b2]), 4).reshape(1, 512)

    in_maps = []
    for c in range(NCORES):
        Xc = X[c * BL : (c + 1) * BL]  # [BL, i, k, d]
        # i = 64h + 8t + 4par + i4 ; partition = 64par + d ; free = (t,i4,k)
        arr = Xc.reshape(BL, 2, 8, 2, 4, 128, 64).transpose(0, 1, 3, 6, 2, 4, 5)
        XT = np.ascontiguousarray(arr).reshape(BL, 2, 128, 4096).astype(bf)
        in_maps.append(
            {
                "XT": XT,
                "A": A[c * BL : (c + 1) * BL].astype(bf),
                "W1D": W1D.astype(bf),
                "W2D": W2D.astype(bf),
                "B1R": B1R,
                "B2R": B2R,
            }
        )
    return in_maps


def _unpack_output(res_list):
    """[BL, j, i, d] per core (maybe bf16) -> full [B, i, j, d] f32."""
    out = np.concatenate([np.asarray(r["OUT"]) for r in res_list], axis=0)
    return np.ascontiguousarray(
        out.astype(np.float32).transpose(0, 2, 1, 3)
    )


def kernel(X, A, W1, b1, W2, b2):
    from concourse import bass_utils

    has_b2 = bool(np.any(np.asarray(b2) != 0.0))
    nc = _get_nc(has_b2)
    in_maps = _pack_inputs(X, A, W1, b1, W2, b2)
    res = bass_utils.run_bass_kernel_spmd(
        nc, in_maps, core_ids=list(range(NCORES))
    )
    return _unpack_output(res.results)
